# revision 1
# baseline (speedup 1.0000x reference)
"""JumpAttention (channel attention, cross-swapped values) on 8 trn2 cores.

Math (per batch b, per head h, hd=64):
  q,k,v = heads(x @ Wq), heads(x @ Wk), heads(x @ Wv)   laid out [hd, N]
  G_h   = q_h k_h^T (contraction over N)  ==  Wq_h^T (x^T x) Wk_h
  attn  = softmax(G / (||q||_clamped ||k||_clamped) * temp, axis=d)
  y1+y2 = x2 @ F1 + x1 @ F2,  F_s = concat_h(Wv_h @ attn_s_h^T)

So the whole kernel is:
  pass 1: S_s = x_s^T x_s  (f32r matmuls, contraction over tokens -> natural
          layout, no transposes), plus cast x->fp16 and PE-transpose tiles
          into SBUF-resident xT (needed as stationary for pass 2)
  interlude (tiny, once): T=S@W, G=Wq^T T, norms via ones-matmul of W*T,
          softmax, F_s = Wv @ attn_s^T
  pass 2: out = x2 @ F1 + x1 @ F2 from SBUF xT, PSUM-accumulated.

Sharding: pure data-parallel over B (B=8 == n_cores). No collectives.
"""

import os
import sys
from contextlib import ExitStack

import numpy as np

for _p in ("/opt/trn_rl_repo",):
    if _p not in sys.path and os.path.isdir(_p):
        sys.path.insert(0, _p)

import concourse.bass as bass  # noqa: E402
import concourse.tile as tile  # noqa: E402
from concourse import bacc, mybir  # noqa: E402
from concourse.bass_utils import run_bass_kernel_spmd  # noqa: E402

B, N_FULL, C = 8, 16384, 256
H, HD = 4, 64
NCORES = 8
TT = 128  # token tile (matmul K)
SLAB = 4  # token tiles per DMA slab

F32 = mybir.dt.float32
F32R = mybir.dt.float32r
F16 = mybir.dt.float16
AF = mybir.ActivationFunctionType
EPS = 1e-12


def _build(n_tokens: int):
    """Build + compile the single-core program (SPMD across 8 cores)."""
    nt = n_tokens // TT
    nslab = n_tokens // (TT * SLAB)
    nc = bacc.Bacc(
        "TRN2", target_bir_lowering=False, debug=False, num_devices=NCORES
    )
    x1 = nc.dram_tensor("x1", [n_tokens, C], F32, kind="ExternalInput").ap()
    x2 = nc.dram_tensor("x2", [n_tokens, C], F32, kind="ExternalInput").ap()
    wblob = nc.dram_tensor("wblob", [128, 3200], F16, kind="ExternalInput").ap()
    tmpd = nc.dram_tensor("tmpd", [128, 4], F32, kind="ExternalInput").ap()
    out = nc.dram_tensor("out", [n_tokens, C], F32, kind="ExternalOutput").ap()

    with tile.TileContext(nc) as tc, ExitStack() as ctx:
        _kernel(ctx, tc, out, [x1, x2], wblob, tmpd, nt, nslab)
    nc.compile()
    return nc


def _kernel(ctx, tc, out, xin, wblob, tmpd, nt, nslab):
    nc = tc.nc
    singles = ctx.enter_context(tc.tile_pool(name="singles", bufs=1))

    # ---- constants / weights to SBUF: ONE blob DMA + one tmp DMA ----
    # blob cols: [0:1024] wkq pair, [1024:1536] wq rows, [1536:2048] wk rows,
    # [2048:2176] identity, [2176:3200] wvt heads (rows 0-63)
    blob_sb = singles.tile([128, 3200], F16, tag="blob", name="blob")
    nc.sync.dma_start(out=blob_sb[:], in_=wblob[:, :])
    wkq_sb = [blob_sb[:, r * 512 : (r + 1) * 512] for r in range(2)]
    wq_sb = [blob_sb[:, 1024 + r * C : 1024 + (r + 1) * C] for r in range(2)]
    wk_sb = [blob_sb[:, 1536 + r * C : 1536 + (r + 1) * C] for r in range(2)]
    ident_sb = blob_sb[:, 2048:2176]
    wvt_sb = [blob_sb[0:HD, 2176 + h * C : 2176 + (h + 1) * C] for h in range(H)]
    tmps_sb = singles.tile([128, 4], F32, tag="tmps", name="tmps")
    nc.sync.dma_start(out=tmps_sb[:], in_=tmpd[:, :])
    tmp_sb = [tmps_sb[:, 2 * s : 2 * s + 2] for s in range(2)]
    ones_col = singles.tile([128, 1], F16, tag="ones_col", name="ones_col")
    nc.vector.memset(ones_col[:], 1.0)
    ones_row = singles.tile([1, 128], F16, tag="ones_row", name="ones_row")
    nc.vector.memset(ones_row[:], 1.0)

    # ---- persistent xT (fp16, [c, tok]) : 2 streams x 2 chunks ----
    xt_pool = ctx.enter_context(tc.tile_pool(name="xt", bufs=1))
    xT = [
        xt_pool.tile([128, 2 * nt * TT], F16, tag=f"xt{s}", name=f"xt{s}")
        for s in range(2)
    ]

    slab_pool = ctx.enter_context(tc.tile_pool(name="slab", bufs=6))
    xf_pool = ctx.enter_context(tc.tile_pool(name="xf", bufs=6))

    S_sb = [
        [singles.tile([128, C], F16, tag=f"ssb{s}{c}", name=f"ssb{s}{c}") for c in range(2)]
        for s in range(2)
    ]

    # ================= pass 1 =================
    with ExitStack() as p1:
        psS = p1.enter_context(tc.tile_pool(name="psS", bufs=1, space="PSUM"))
        psT = p1.enter_context(tc.tile_pool(name="psT", bufs=4, space="PSUM"))
        S_ps = [
            [psS.tile([128, C], F32, tag=f"s{s}{c}", name=f"s{s}{c}") for c in range(2)]
            for s in range(2)
        ]
        for si in range(nslab):
            slabs = []
            for s in range(2):
                sl = slab_pool.tile([128, SLAB * C], F32, tag="slab", name="slab")
                nc.sync.dma_start(
                    out=sl[:].rearrange("p (t c) -> p t c", t=SLAB),
                    in_=xin[s][
                        si * SLAB * TT : (si + 1) * SLAB * TT, :
                    ].rearrange("(t p) c -> p t c", p=128),
                )
                slabs.append(sl)
            for t in range(SLAB):
                ti = si * SLAB + t
                for s in range(2):
                    x_t = slabs[s][:, t * C : (t + 1) * C]
                    # cast fp32 -> fp16 on the otherwise-idle GpSimd
                    xf = xf_pool.tile([128, C], F16, tag="xf", name="xf")
                    if s == 0:
                        nc.gpsimd.tensor_copy(xf[:], x_t)
                    else:
                        nc.vector.tensor_copy(xf[:], x_t)
                    # S += xf^T @ xf   (f16, moving N=256)
                    for c0 in range(2):
                        nc.tensor.matmul(
                            S_ps[s][c0][:],
                            lhsT=xf[:, c0 * 128 : (c0 + 1) * 128],
                            rhs=xf[:],
                            start=(ti == 0),
                            stop=(ti == nt - 1),
                            skip_group_check=True,
                        )
                    # PE-transpose both chunks into one psum tile, then one
                    # copy into resident xT (dest view: both chunk blocks)
                    tp = psT.tile([128, C], F16, tag="tp", name="tp")
                    for c0 in range(2):
                        nc.tensor.transpose(
                            tp[:, c0 * 128 : (c0 + 1) * 128],
                            xf[:, c0 * 128 : (c0 + 1) * 128],
                            ident_sb,
                        )
                    dst = xT[s][:].rearrange("p (c n) -> p c n", c=2)[
                        :, :, ti * TT : (ti + 1) * TT
                    ]
                    tsrc = tp[:].rearrange("p (c n) -> p c n", c=2)
                    if s == 0 or ti >= nt - 8:
                        nc.vector.tensor_copy(dst, tsrc)
                    else:
                        nc.scalar.activation(dst, tsrc, AF.Copy)
        # S -> SBUF (still inside psS scope); split DVE/ACT
        for s in range(2):
            for c0 in range(2):
                if s == 0:
                    nc.vector.tensor_copy(S_sb[s][c0][:], S_ps[s][c0][:])
                else:
                    nc.scalar.activation(S_sb[s][c0][:], S_ps[s][c0][:], AF.Copy)

    # ================= interlude =================
    # Staged across both streams so the two dependency chains overlap and
    # ACT function-table loads (Sqrt/Exp) cluster instead of thrashing.
    F_sb = [
        [singles.tile([128, C], F16, tag=f"f{s}{jc}", name=f"f{s}{jc}") for jc in range(2)]
        for s in range(2)
    ]
    with ExitStack() as il:
        big = il.enter_context(tc.tile_pool(name="ilbig", bufs=6, space="PSUM"))
        small = il.enter_context(
            tc.tile_pool(name="ilsmall", bufs=2, space="PSUM")
        )
        sb = il.enter_context(tc.tile_pool(name="ilsb", bufs=1))

        warm = sb.tile([1, 1], F32, tag="warm", name="warm")

        # -- stage A: [T_k | T_q] = S @ [Wk | Wq]  (f16 matmuls, N=512) --
        t_sb = {}
        for s in range(2):
            for ic in range(2):
                tp = big.tile([128, 2 * C], F32, tag="ilbig", name="ilbig")
                for jc in range(2):
                    nc.tensor.matmul(
                        tp[:],
                        lhsT=S_sb[s][jc][:, ic * 128 : (ic + 1) * 128],
                        rhs=wkq_sb[jc],
                        start=(jc == 0),
                        stop=(jc == 1),
                    )
                st = sb.tile([128, 2 * C], F16, tag=f"t{ic}_{s}", name=f"t{ic}_{s}")
                if s == 0:
                    nc.vector.tensor_copy(st[:, 0:C], tp[:, 0:C])
                    nc.vector.tensor_copy(st[:, C : 2 * C], tp[:, C : 2 * C])
                else:
                    nc.scalar.activation(st[:], tp[:], AF.Copy)
                t_sb["k", s, ic] = st[:, 0:C]
                t_sb["q", s, ic] = st[:, C : 2 * C]

        # -- stage B: G = Wq^T @ T_k (psum) ; U = W * T (DVE) --
        g_ps = {}
        for s in range(2):
            for cc in range(2):
                g = big.tile([128, C], F32, tag="ilbig", name="ilbig")
                for ic in range(2):
                    nc.tensor.matmul(
                        g[:],
                        lhsT=wq_sb[ic][:, cc * 128 : (cc + 1) * 128],
                        rhs=t_sb["k", s, ic],
                        start=(ic == 0),
                        stop=(ic == 1),
                    )
                g_ps[s, cc] = g
        u_sb = {}
        for s in range(2):
            for nm in ("q", "k"):
                w_sb = wq_sb if nm == "q" else wk_sb
                for ic in range(2):
                    u = sb.tile([128, C], F16, tag=f"u{nm}{ic}_{s}", name=f"u{nm}{ic}_{s}")
                    nc.vector.tensor_mul(u[:], w_sb[ic], t_sb[nm, s, ic])
                    u_sb[nm, s, ic] = u

        # preload Sqrt table: emitted after every interlude ACT Copy so the
        # in-order ACT queue does Copy* -> Sqrt* with a single table load
        nc.scalar.activation(warm[:], S_sb[1][1][0:1, 0:1], AF.Sqrt)

        # -- stage C: norm matmuls + invq/invk (Sqrt clustered on ACT) --
        nq_ps, nk_ps = {}, {}
        for s in range(2):
            for cc in range(2):
                nq = small.tile([128, 1], F32, tag="ilsmall", name="ilsmall")
                for ic in range(2):
                    nc.tensor.matmul(
                        nq[:],
                        lhsT=u_sb["q", s, ic][:, cc * 128 : (cc + 1) * 128],
                        rhs=ones_col[:],
                        start=(ic == 0),
                        stop=(ic == 1),
                    )
                nq_ps[s, cc] = nq
            nk = small.tile([1, C], F32, tag="ilsmall", name="ilsmall_r")
            for ic in range(2):
                nc.tensor.matmul(
                    nk[:],
                    lhsT=ones_col[:],
                    rhs=u_sb["k", s, ic],
                    start=(ic == 0),
                    stop=(ic == 1),
                )
            nk_ps[s] = nk
        invq, ik16 = {}, {}
        for s in range(2):
            for cc in range(2):
                iq = sb.tile([128, 1], F32, tag=f"invq{cc}_{s}", name=f"invq{cc}_{s}")
                nc.scalar.activation(iq[:], nq_ps[s, cc][:], AF.Sqrt)
                invq[s, cc] = iq
            ik = sb.tile([1, C], F32, tag=f"invk_{s}", name=f"invk_{s}")
            nc.scalar.activation(ik[:], nk_ps[s][:], AF.Sqrt)
            ik16[s] = ik
        for s in range(2):
            for cc in range(2):
                iq = invq[s, cc]
                nc.vector.tensor_scalar_max(iq[:], iq[:], EPS)
                nc.vector.reciprocal(iq[:], iq[:])
                nc.vector.tensor_mul(iq[:], iq[:], tmp_sb[s][:, cc : cc + 1])
            ik = ik16[s]
            nc.vector.tensor_scalar_max(ik[:], ik[:], EPS)
            nc.vector.reciprocal(ik[:], ik[:])
            ikf = sb.tile([1, C], F16, tag=f"invk16_{s}", name=f"invk16_{s}")
            nc.vector.tensor_copy(ikf[:], ik[:])
            ik16[s] = ikf

        # preload Exp table while stage D runs on DVE
        nc.scalar.activation(warm[:], invq[0, 0][0:1, :], AF.Exp)

        # -- stage D: invk broadcast + logits assembly + row max --
        bc_ps, lp_sb, nm_sb = {}, {}, {}
        for s in range(2):
            bc = big.tile([128, C], F32, tag="ilbig", name="ilbig")
            nc.tensor.matmul(
                bc[:], lhsT=ones_row[:], rhs=ik16[s][:], start=True, stop=True
            )
            bc_ps[s] = bc
        for s in range(2):
            for cc in range(2):
                lp = sb.tile([128, HD], F32, tag=f"lp{cc}_{s}", name=f"lp{cc}_{s}")
                for half in range(2):
                    h = 2 * cc + half
                    rs = slice(half * 64, (half + 1) * 64)
                    cs = slice(h * 64, (h + 1) * 64)
                    nc.vector.tensor_scalar_mul(
                        lp[rs, :], g_ps[s, cc][rs, cs], invq[s, cc][rs, :]
                    )
                    nc.vector.tensor_mul(lp[rs, :], lp[rs, :], bc_ps[s][rs, cs])
                nmt = sb.tile([128, 1], F32, tag=f"nm{cc}_{s}", name=f"nm{cc}_{s}")
                nc.vector.tensor_reduce(
                    nmt[:], lp[:], axis=mybir.AxisListType.X,
                    op=mybir.AluOpType.max, negate=True,
                )
                lp_sb[s, cc] = lp
                nm_sb[s, cc] = nmt

        # -- stage E: Exp (clustered) + normalize --
        attn16 = {}
        for s in range(2):
            for cc in range(2):
                pexp = sb.tile([128, HD], F32, tag=f"pexp{cc}_{s}", name=f"pexp{cc}_{s}")
                sm = sb.tile([128, 1], F32, tag=f"sm{cc}_{s}", name=f"sm{cc}_{s}")
                nc.scalar.activation(
                    pexp[:], lp_sb[s, cc][:], AF.Exp,
                    bias=nm_sb[s, cc][:], accum_out=sm[:],
                )
                attn16[s, cc] = (pexp, sm)
        for s in range(2):
            for cc in range(2):
                pexp, sm = attn16[s, cc]
                nc.vector.reciprocal(sm[:], sm[:])
                a16 = sb.tile([128, HD], F16, tag=f"a16{cc}_{s}", name=f"a16{cc}_{s}")
                nc.vector.tensor_scalar_mul(a16[:], pexp[:], sm[:])
                attn16[s, cc] = a16

        # -- stage F: attn^T + F = concat_h(Wv_h @ attn_h^T) --
        at_sb = {}
        for s in range(2):
            for cc in range(2):
                atp = small.tile([HD, 128], F16, tag="ilsmall", name="ilsmall_t")
                nc.tensor.transpose(atp[:], attn16[s, cc][:], ident_sb)
                at = sb.tile([HD, 128], F16, tag=f"at{cc}_{s}", name=f"at{cc}_{s}")
                nc.vector.tensor_copy(at[:], atp[:])
                at_sb[s, cc] = at
        for s in range(2):
            for jc in range(2):
                fp = big.tile([128, C], F32, tag="ilbig", name="ilbig")
                for h in range(H):
                    cc, half = divmod(h, 2)
                    nc.tensor.matmul(
                        fp[:, h * 64 : (h + 1) * 64],
                        lhsT=wvt_sb[h][:, jc * 128 : (jc + 1) * 128],
                        rhs=at_sb[s, cc][:, half * 64 : (half + 1) * 64],
                        start=True,
                        stop=True,
                    )
                if s == 0:
                    nc.vector.tensor_copy(F_sb[s][jc][:], fp[:])
                else:
                    nc.scalar.activation(F_sb[s][jc][:], fp[:], AF.Copy)

    # ================= pass 2 =================
    with ExitStack() as p2:
        psO = p2.enter_context(tc.tile_pool(name="psO", bufs=6, space="PSUM"))
        opool = p2.enter_context(tc.tile_pool(name="opool", bufs=4))
        pairs = [(xT[1], F_sb[0]), (xT[0], F_sb[1])]  # y1 = x2@F1, y2 = x1@F2
        nn = nt * TT
        for si in range(nslab):
            osl = opool.tile([128, SLAB * C], F32, tag="osl", name="osl")
            for t in range(SLAB):
                ti = si * SLAB + t
                op = psO.tile([128, C], F32, tag="op", name="op")
                idx = 0
                for xts, fs in pairs:
                    for jc in range(2):
                        nc.tensor.matmul(
                            op[:],
                            lhsT=xts[
                                :, jc * nn + ti * TT : jc * nn + (ti + 1) * TT
                            ],
                            rhs=fs[jc][:],
                            start=(idx == 0),
                            stop=(idx == 3),
                        )
                        idx += 1
                if t % 2 == 0:
                    nc.vector.tensor_copy(osl[:, t * C : (t + 1) * C], op[:])
                else:
                    nc.scalar.activation(
                        osl[:, t * C : (t + 1) * C], op[:], AF.Copy
                    )
            nc.sync.dma_start(
                out=out[si * SLAB * TT : (si + 1) * SLAB * TT, :].rearrange(
                    "(t p) c -> p t c", p=128
                ),
                in_=osl[:].rearrange("p (t c) -> p t c", t=SLAB),
            )


def _host_prep(w_qkv, temperature, temperature2):
    w = np.asarray(w_qkv, dtype=np.float32)
    wq = w[:, 0:C].astype(np.float16)
    wk = w[:, C : 2 * C].astype(np.float16)
    wvt = np.ascontiguousarray(w[:, 2 * C : 3 * C].T.reshape(H, HD, C)).astype(
        np.float16
    )
    blob = np.zeros((128, 3200), dtype=np.float16)
    for r in range(2):
        blob[:, r * 512 : r * 512 + C] = wk[r * 128 : (r + 1) * 128, :]
        blob[:, r * 512 + C : (r + 1) * 512] = wq[r * 128 : (r + 1) * 128, :]
        blob[:, 1024 + r * C : 1024 + (r + 1) * C] = wq[
            r * 128 : (r + 1) * 128, :
        ]
        blob[:, 1536 + r * C : 1536 + (r + 1) * C] = wk[
            r * 128 : (r + 1) * 128, :
        ]
    blob[:, 2048:2176] = np.eye(128, dtype=np.float16)
    for h in range(H):
        blob[0:HD, 2176 + h * C : 2176 + (h + 1) * C] = wvt[h]
    tmp = []
    for tarr in (temperature, temperature2):
        t = np.asarray(tarr, dtype=np.float32).reshape(H)
        tmp.append(
            np.stack(
                [np.repeat(t[[0, 1]], 64), np.repeat(t[[2, 3]], 64)], axis=1
            ).astype(np.float32)
        )
    tmpd = np.concatenate(tmp, axis=1).astype(np.float32)
    return blob, tmpd


_NC_CACHE = {}
LAST_RESULT = None


def _get_nc(n_tokens):
    if n_tokens not in _NC_CACHE:
        _NC_CACHE[n_tokens] = _build(n_tokens)
    return _NC_CACHE[n_tokens]


def kernel(x1, x2, w_qkv, temperature, temperature2):
    global LAST_RESULT
    x1 = np.asarray(x1, dtype=np.float32)
    x2 = np.asarray(x2, dtype=np.float32)
    b, n, c = x1.shape
    assert c == C and b == NCORES, (b, n, c)
    wblob, tmpd = _host_prep(w_qkv, temperature, temperature2)
    nc = _get_nc(n)
    in_maps = [
        {
            "x1": np.ascontiguousarray(x1[i]),
            "x2": np.ascontiguousarray(x2[i]),
            "wblob": wblob,
            "tmpd": tmpd,
        }
        for i in range(NCORES)
    ]
    res = run_bass_kernel_spmd(nc, in_maps, list(range(NCORES)))
    LAST_RESULT = res
    return np.stack([r["out"] for r in res.results]).reshape(b, n, c)



# revision 2
# speedup vs baseline: 1.5741x; 1.5741x over previous
"""JumpAttention (channel attention, cross-swapped values) on 8 trn2 cores.

Math (per batch b, per head h, hd=64):
  q,k,v = heads(x @ Wq), heads(x @ Wk), heads(x @ Wv)   laid out [hd, N]
  G_h   = q_h k_h^T (contraction over N)  ==  Wq_h^T (x^T x) Wk_h
  attn  = softmax(G / (||q||_clamped ||k||_clamped) * temp, axis=d)
  y1+y2 = x2 @ F1 + x1 @ F2,  F_s = concat_h(Wv_h @ attn_s_h^T)

Kernel structure (v2 — fp8 pass-1 + host-transposed x for pass-2):
  pass 1: S_s = x_s^T x_s from an fp8(e4m3) copy of x using DoubleRow
          matmuls (2 tokens packed per partition -> 2x fp16 PE rate).
          S is symmetric, so the lower-left 128x128 block is never
          computed; the interlude reads it through the transposed
          upper-right block (lhsT substitution).
  interlude (tiny, once): T=S@W, G=Wq^T T, norms via ones-matmul of
          W*T, softmax, F_s = Wv @ attn_s^T
  pass 2: out = x2 @ F1 + x1 @ F2 with lhsT streamed from a HOST-
          TRANSPOSED fp16 x^T in DRAM (no PE transposes at all),
          fp16 output written back (host upcasts to f32).

  DMA traffic/core: 8MB (x fp8) + 16MB (x^T fp16) + 8MB (out) = 32MB,
  all on one in-order SP queue ordered x8 -> xt -> out so the DMA
  engines (the bottleneck resource) never idle.

Sharding: pure data-parallel over B (B=8 == n_cores). No collectives.
"""

import os
import sys
from contextlib import ExitStack

import numpy as np
import ml_dtypes

for _p in ("/opt/trn_rl_repo",):
    if _p not in sys.path and os.path.isdir(_p):
        sys.path.insert(0, _p)

import concourse.bass as bass  # noqa: E402
import concourse.tile as tile  # noqa: E402
from concourse import bacc, mybir  # noqa: E402
from concourse.bass_utils import run_bass_kernel_spmd  # noqa: E402

B, N_FULL, C = 8, 16384, 256
H, HD = 4, 64
NCORES = 8

F32 = mybir.dt.float32
F16 = mybir.dt.float16
F8 = mybir.dt.float8e4
AF = mybir.ActivationFunctionType
DR = mybir.MatmulPerfMode.DoubleRow
EPS = 1e-12

SLABT = 4096  # tokens per x8 slab (1MB fp8)
WINT = 4096  # tokens per xt window (4 x 1MB fp16 tiles)
OSLT = 2048  # tokens per out slab (1MB fp16)


def _build(n_tokens: int):
    """Build + compile the single-core program (SPMD across 8 cores)."""
    nc = bacc.Bacc(
        "TRN2", target_bir_lowering=False, debug=False, num_devices=NCORES
    )
    x8 = [
        nc.dram_tensor(f"x{s + 1}f8", [n_tokens, C], F8, kind="ExternalInput").ap()
        for s in range(2)
    ]
    xt = [
        nc.dram_tensor(f"xt{s + 1}", [C, n_tokens], F16, kind="ExternalInput").ap()
        for s in range(2)
    ]
    wblob = nc.dram_tensor("wblob", [128, 3200], F16, kind="ExternalInput").ap()
    tmpd = nc.dram_tensor("tmpd", [128, 4], F32, kind="ExternalInput").ap()
    out = nc.dram_tensor("out", [n_tokens, C], F16, kind="ExternalOutput").ap()

    with tile.TileContext(nc) as tc, ExitStack() as ctx:
        _kernel(ctx, tc, out, x8, xt, wblob, tmpd, n_tokens)
    nc.compile()
    return nc


def _kernel(ctx, tc, out, x8in, xtin, wblob, tmpd, nt):
    nc = tc.nc
    nslab = nt // SLABT
    nwin = nt // WINT
    singles = ctx.enter_context(tc.tile_pool(name="singles", bufs=1))

    # ---- constants / weights to SBUF: ONE blob DMA + one tmp DMA ----
    # blob cols: [0:1024] wkq pair, [1024:1536] wq rows, [1536:2048] wk rows,
    # [2048:2176] identity, [2176:3200] wvt heads (rows 0-63)
    blob_sb = singles.tile([128, 3200], F16, tag="blob", name="blob")
    nc.sync.dma_start(out=blob_sb[:], in_=wblob[:, :])
    wkq_sb = [blob_sb[:, r * 512 : (r + 1) * 512] for r in range(2)]
    wq_sb = [blob_sb[:, 1024 + r * C : 1024 + (r + 1) * C] for r in range(2)]
    wk_sb = [blob_sb[:, 1536 + r * C : 1536 + (r + 1) * C] for r in range(2)]
    ident_sb = blob_sb[:, 2048:2176]
    wvt_sb = [blob_sb[0:HD, 2176 + h * C : 2176 + (h + 1) * C] for h in range(H)]
    tmps_sb = singles.tile([128, 4], F32, tag="tmps", name="tmps")
    nc.sync.dma_start(out=tmps_sb[:], in_=tmpd[:, :])
    tmp_sb = [tmps_sb[:, 2 * s : 2 * s + 2] for s in range(2)]
    ones_col = singles.tile([128, 1], F16, tag="ones_col", name="ones_col")
    nc.vector.memset(ones_col[:], 1.0)
    ones_row = singles.tile([1, 128], F16, tag="ones_row", name="ones_row")
    nc.vector.memset(ones_row[:], 1.0)

    S_sb = [
        [singles.tile([128, C], F16, tag=f"ssb{s}{c}", name=f"ssb{s}{c}") for c in range(2)]
        for s in range(2)
    ]

    # ================= pass 1: S = x^T x via fp8 DoubleRow =================
    # slab layout: partition p holds SLABT/128 consecutive tokens; a pair
    # tile tau contracts tokens {p*(SLABT/128) + 2*tau + j} over (p, j).
    x8_pool = ctx.enter_context(tc.tile_pool(name="x8", bufs=4))
    tpp = SLABT // 128  # tokens per partition per slab
    npair = tpp // 2  # pair tiles per slab
    # S row-chunk widths (symmetry: lower-left 128-block never computed)
    # chunk m0=0..3 covers S rows [m0*64, m0*64+64); rhs col range below.
    chunk_cols = [(0, 256), (0, 256), (128, 128), (128, 128)]

    with ExitStack() as p1:
        psS = p1.enter_context(tc.tile_pool(name="psS", bufs=1, space="PSUM"))
        S_ps = [
            [psS.tile([128, C], F32, tag=f"s{s}{c}", name=f"s{s}{c}") for c in range(2)]
            for s in range(2)
        ]
        for s in range(2):
            for si in range(nslab):
                sl = x8_pool.tile([128, SLABT * C // 128], F8, tag="x8slab", name="x8slab")
                nc.sync.dma_start(
                    out=sl[:].rearrange("p (q c) -> p q c", q=tpp),
                    in_=x8in[s][si * SLABT : (si + 1) * SLABT, :].rearrange(
                        "(p q) c -> p q c", p=128
                    ),
                )
                for t in range(npair):
                    ti = si * npair + t
                    pv = sl[:, t * 2 * C : (t + 1) * 2 * C].rearrange(
                        "p (j c) -> p j c", j=2
                    )
                    for m0 in range(4):
                        c0, cw = chunk_cols[m0]
                        nc.tensor.matmul(
                            S_ps[s][m0 // 2][(m0 % 2) * 64 : (m0 % 2) * 64 + 64, c0 : c0 + cw],
                            lhsT=pv[:, :, m0 * 64 : (m0 + 1) * 64],
                            rhs=pv[:, :, c0 : c0 + cw],
                            start=(ti == 0),
                            stop=(ti == nslab * npair - 1),
                            perf_mode=DR,
                            skip_group_check=True,
                        )
        # S -> SBUF. tile0 rows 0:128 cols 0:256 full; tile1 cols 128:256 only.
        for s in range(2):
            if s == 0:
                nc.vector.tensor_copy(S_sb[s][0][:], S_ps[s][0][:])
                nc.vector.tensor_copy(
                    S_sb[s][1][:, 128:256], S_ps[s][1][:, 128:256]
                )
            else:
                nc.scalar.activation(S_sb[s][0][:], S_ps[s][0][:], AF.Copy)
                nc.scalar.activation(
                    S_sb[s][1][:, 128:256], S_ps[s][1][:, 128:256], AF.Copy
                )

    # xt windows 0..nwin-2 prefetch (w3 issued mid-pass-2, after first osl)
    xt_sb = {}
    for s in range(2):
        for jc in range(2):
            for w in range(nwin):
                xt_sb[s, jc, w] = singles.tile(
                    [128, WINT], F16, tag=f"xt{s}{jc}{w}", name=f"xt{s}{jc}{w}"
                )

    def issue_xt_window(w):
        for s in range(2):
            for jc in range(2):
                nc.sync.dma_start(
                    out=xt_sb[s, jc, w][:],
                    in_=xtin[s][jc * 128 : (jc + 1) * 128, w * WINT : (w + 1) * WINT],
                )

    for w in range(nwin - 1):
        issue_xt_window(w)

    # ================= interlude =================
    # lhsT substitution for the never-computed lower-left S block:
    # S[128:256, 0:128] == S[0:128, 128:256]^T, and matmul transposes lhsT.
    def s_lhsT(s, jc, ic, cc_lo, cc_hi):
        if jc == 1 and ic == 0:
            return S_sb[s][0][:, 128 + cc_lo : 128 + cc_hi]
        return S_sb[s][jc][:, ic * 128 + cc_lo : ic * 128 + cc_hi]

    F_sb = [
        [singles.tile([128, C], F16, tag=f"f{s}{jc}", name=f"f{s}{jc}") for jc in range(2)]
        for s in range(2)
    ]
    with ExitStack() as il:
        big = il.enter_context(tc.tile_pool(name="ilbig", bufs=6, space="PSUM"))
        small = il.enter_context(
            tc.tile_pool(name="ilsmall", bufs=2, space="PSUM")
        )
        sb = il.enter_context(tc.tile_pool(name="ilsb", bufs=1))

        warm = sb.tile([1, 1], F32, tag="warm", name="warm")

        # -- stage A: [T_k | T_q] = S @ [Wk | Wq]  (f16 matmuls, N=512) --
        t_sb = {}
        for s in range(2):
            for ic in range(2):
                tp = big.tile([128, 2 * C], F32, tag="ilbig", name="ilbig")
                for jc in range(2):
                    nc.tensor.matmul(
                        tp[:],
                        lhsT=s_lhsT(s, jc, ic, 0, 128),
                        rhs=wkq_sb[jc],
                        start=(jc == 0),
                        stop=(jc == 1),
                    )
                st = sb.tile([128, 2 * C], F16, tag=f"t{ic}_{s}", name=f"t{ic}_{s}")
                if s == 0:
                    nc.vector.tensor_copy(st[:, 0:C], tp[:, 0:C])
                    nc.vector.tensor_copy(st[:, C : 2 * C], tp[:, C : 2 * C])
                else:
                    nc.scalar.activation(st[:], tp[:], AF.Copy)
                t_sb["k", s, ic] = st[:, 0:C]
                t_sb["q", s, ic] = st[:, C : 2 * C]

        # -- stage B: G = Wq^T @ T_k (psum) ; U = W * T (DVE) --
        g_ps = {}
        for s in range(2):
            for cc in range(2):
                g = big.tile([128, C], F32, tag="ilbig", name="ilbig")
                for ic in range(2):
                    nc.tensor.matmul(
                        g[:],
                        lhsT=wq_sb[ic][:, cc * 128 : (cc + 1) * 128],
                        rhs=t_sb["k", s, ic],
                        start=(ic == 0),
                        stop=(ic == 1),
                    )
                g_ps[s, cc] = g
        u_sb = {}
        for s in range(2):
            for nm in ("q", "k"):
                w_sb = wq_sb if nm == "q" else wk_sb
                for ic in range(2):
                    u = sb.tile([128, C], F16, tag=f"u{nm}{ic}_{s}", name=f"u{nm}{ic}_{s}")
                    nc.vector.tensor_mul(u[:], w_sb[ic], t_sb[nm, s, ic])
                    u_sb[nm, s, ic] = u

        # preload Sqrt table: emitted after every interlude ACT Copy so the
        # in-order ACT queue does Copy* -> Sqrt* with a single table load
        nc.scalar.activation(warm[:], S_sb[1][1][0:1, 128:129], AF.Sqrt)

        # -- stage C: norm matmuls + invq/invk (Sqrt clustered on ACT) --
        nq_ps, nk_ps = {}, {}
        for s in range(2):
            for cc in range(2):
                nq = small.tile([128, 1], F32, tag="ilsmall", name="ilsmall")
                for ic in range(2):
                    nc.tensor.matmul(
                        nq[:],
                        lhsT=u_sb["q", s, ic][:, cc * 128 : (cc + 1) * 128],
                        rhs=ones_col[:],
                        start=(ic == 0),
                        stop=(ic == 1),
                    )
                nq_ps[s, cc] = nq
            nk = small.tile([1, C], F32, tag="ilsmall", name="ilsmall_r")
            for ic in range(2):
                nc.tensor.matmul(
                    nk[:],
                    lhsT=ones_col[:],
                    rhs=u_sb["k", s, ic],
                    start=(ic == 0),
                    stop=(ic == 1),
                )
            nk_ps[s] = nk
        invq, ik16 = {}, {}
        for s in range(2):
            for cc in range(2):
                iq = sb.tile([128, 1], F32, tag=f"invq{cc}_{s}", name=f"invq{cc}_{s}")
                nc.scalar.activation(iq[:], nq_ps[s, cc][:], AF.Sqrt)
                invq[s, cc] = iq
            ik = sb.tile([1, C], F32, tag=f"invk_{s}", name=f"invk_{s}")
            nc.scalar.activation(ik[:], nk_ps[s][:], AF.Sqrt)
            ik16[s] = ik
        for s in range(2):
            for cc in range(2):
                iq = invq[s, cc]
                nc.vector.tensor_scalar_max(iq[:], iq[:], EPS)
                nc.vector.reciprocal(iq[:], iq[:])
                nc.vector.tensor_mul(iq[:], iq[:], tmp_sb[s][:, cc : cc + 1])
            ik = ik16[s]
            nc.vector.tensor_scalar_max(ik[:], ik[:], EPS)
            nc.vector.reciprocal(ik[:], ik[:])
            ikf = sb.tile([1, C], F16, tag=f"invk16_{s}", name=f"invk16_{s}")
            nc.vector.tensor_copy(ikf[:], ik[:])
            ik16[s] = ikf

        # preload Exp table while stage D runs on DVE
        nc.scalar.activation(warm[:], invq[0, 0][0:1, :], AF.Exp)

        # -- stage D: invk broadcast + logits assembly + row max --
        bc_ps, lp_sb, nm_sb = {}, {}, {}
        for s in range(2):
            bc = big.tile([128, C], F32, tag="ilbig", name="ilbig")
            nc.tensor.matmul(
                bc[:], lhsT=ones_row[:], rhs=ik16[s][:], start=True, stop=True
            )
            bc_ps[s] = bc
        for s in range(2):
            for cc in range(2):
                lp = sb.tile([128, HD], F32, tag=f"lp{cc}_{s}", name=f"lp{cc}_{s}")
                for half in range(2):
                    h = 2 * cc + half
                    rs = slice(half * 64, (half + 1) * 64)
                    cs = slice(h * 64, (h + 1) * 64)
                    nc.vector.tensor_scalar_mul(
                        lp[rs, :], g_ps[s, cc][rs, cs], invq[s, cc][rs, :]
                    )
                    nc.vector.tensor_mul(lp[rs, :], lp[rs, :], bc_ps[s][rs, cs])
                nmt = sb.tile([128, 1], F32, tag=f"nm{cc}_{s}", name=f"nm{cc}_{s}")
                nc.vector.tensor_reduce(
                    nmt[:], lp[:], axis=mybir.AxisListType.X,
                    op=mybir.AluOpType.max, negate=True,
                )
                lp_sb[s, cc] = lp
                nm_sb[s, cc] = nmt

        # -- stage E: Exp (clustered) + normalize --
        attn16 = {}
        for s in range(2):
            for cc in range(2):
                pexp = sb.tile([128, HD], F32, tag=f"pexp{cc}_{s}", name=f"pexp{cc}_{s}")
                sm = sb.tile([128, 1], F32, tag=f"sm{cc}_{s}", name=f"sm{cc}_{s}")
                nc.scalar.activation(
                    pexp[:], lp_sb[s, cc][:], AF.Exp,
                    bias=nm_sb[s, cc][:], accum_out=sm[:],
                )
                attn16[s, cc] = (pexp, sm)
        for s in range(2):
            for cc in range(2):
                pexp, sm = attn16[s, cc]
                nc.vector.reciprocal(sm[:], sm[:])
                a16 = sb.tile([128, HD], F16, tag=f"a16{cc}_{s}", name=f"a16{cc}_{s}")
                nc.vector.tensor_scalar_mul(a16[:], pexp[:], sm[:])
                attn16[s, cc] = a16

        # -- stage F: attn^T + F = concat_h(Wv_h @ attn_h^T) --
        at_sb = {}
        for s in range(2):
            for cc in range(2):
                atp = small.tile([HD, 128], F16, tag="ilsmall", name="ilsmall_t")
                nc.tensor.transpose(atp[:], attn16[s, cc][:], ident_sb)
                at = sb.tile([HD, 128], F16, tag=f"at{cc}_{s}", name=f"at{cc}_{s}")
                nc.vector.tensor_copy(at[:], atp[:])
                at_sb[s, cc] = at
        for s in range(2):
            for jc in range(2):
                fp = big.tile([128, C], F32, tag="ilbig", name="ilbig")
                for h in range(H):
                    cc, half = divmod(h, 2)
                    nc.tensor.matmul(
                        fp[:, h * 64 : (h + 1) * 64],
                        lhsT=wvt_sb[h][:, jc * 128 : (jc + 1) * 128],
                        rhs=at_sb[s, cc][:, half * 64 : (half + 1) * 64],
                        start=True,
                        stop=True,
                    )
                if s == 0:
                    nc.vector.tensor_copy(F_sb[s][jc][:], fp[:])
                else:
                    nc.scalar.activation(F_sb[s][jc][:], fp[:], AF.Copy)

    # ================= pass 2: out = x2@F1 + x1@F2 =================
    with ExitStack() as p2:
        psO = p2.enter_context(tc.tile_pool(name="psO", bufs=6, space="PSUM"))
        opool = p2.enter_context(tc.tile_pool(name="opool", bufs=3))
        # pairs: (xt stream index, F): y1 = x2@F1, y2 = x1@F2
        pairs = [(1, F_sb[0]), (0, F_sb[1])]
        nosl = nt // OSLT
        tpo = OSLT // 128  # psum tiles per out slab
        issued_last = False
        for oi in range(nosl):
            w = oi * OSLT // WINT
            osl = opool.tile([128, tpo * C], F16, tag="osl", name="osl")
            for t in range(tpo):
                tw = (oi * OSLT + t * 128 - w * WINT) // 128  # tile within window
                op = psO.tile([128, C], F32, tag="op", name="op")
                idx = 0
                for sx, fs in pairs:
                    for jc in range(2):
                        nc.tensor.matmul(
                            op[:],
                            lhsT=xt_sb[sx, jc, w][:, tw * 128 : (tw + 1) * 128],
                            rhs=fs[jc][:],
                            start=(idx == 0),
                            stop=(idx == 3),
                        )
                        idx += 1
                if t % 2 == 0:
                    nc.vector.tensor_copy(osl[:, t * C : (t + 1) * C], op[:])
                else:
                    nc.scalar.activation(
                        osl[:, t * C : (t + 1) * C], op[:], AF.Copy
                    )
            nc.sync.dma_start(
                out=out[oi * OSLT : (oi + 1) * OSLT, :].rearrange(
                    "(t p) c -> p t c", p=128
                ),
                in_=osl[:].rearrange("p (t c) -> p t c", t=tpo),
            )
            if not issued_last:
                # queue the final xt window behind the first out slab
                issue_xt_window(nwin - 1)
                issued_last = True


def _host_prep(w_qkv, temperature, temperature2):
    w = np.asarray(w_qkv, dtype=np.float32)
    wq = w[:, 0:C].astype(np.float16)
    wk = w[:, C : 2 * C].astype(np.float16)
    wvt = np.ascontiguousarray(w[:, 2 * C : 3 * C].T.reshape(H, HD, C)).astype(
        np.float16
    )
    blob = np.zeros((128, 3200), dtype=np.float16)
    for r in range(2):
        blob[:, r * 512 : r * 512 + C] = wk[r * 128 : (r + 1) * 128, :]
        blob[:, r * 512 + C : (r + 1) * 512] = wq[r * 128 : (r + 1) * 128, :]
        blob[:, 1024 + r * C : 1024 + (r + 1) * C] = wq[
            r * 128 : (r + 1) * 128, :
        ]
        blob[:, 1536 + r * C : 1536 + (r + 1) * C] = wk[
            r * 128 : (r + 1) * 128, :
        ]
    blob[:, 2048:2176] = np.eye(128, dtype=np.float16)
    for h in range(H):
        blob[0:HD, 2176 + h * C : 2176 + (h + 1) * C] = wvt[h]
    tmp = []
    for tarr in (temperature, temperature2):
        t = np.asarray(tarr, dtype=np.float32).reshape(H)
        tmp.append(
            np.stack(
                [np.repeat(t[[0, 1]], 64), np.repeat(t[[2, 3]], 64)], axis=1
            ).astype(np.float32)
        )
    tmpd = np.concatenate(tmp, axis=1).astype(np.float32)
    return blob, tmpd


_NC_CACHE = {}
LAST_RESULT = None


def _get_nc(n_tokens):
    if n_tokens not in _NC_CACHE:
        _NC_CACHE[n_tokens] = _build(n_tokens)
    return _NC_CACHE[n_tokens]


def kernel(x1, x2, w_qkv, temperature, temperature2):
    global LAST_RESULT
    x1 = np.asarray(x1, dtype=np.float32)
    x2 = np.asarray(x2, dtype=np.float32)
    b, n, c = x1.shape
    assert c == C and b == NCORES, (b, n, c)
    wblob, tmpd = _host_prep(w_qkv, temperature, temperature2)
    nc = _get_nc(n)
    in_maps = []
    for i in range(NCORES):
        m = {"wblob": wblob, "tmpd": tmpd}
        for s, x in enumerate((x1, x2)):
            m[f"x{s + 1}f8"] = np.ascontiguousarray(x[i]).astype(
                ml_dtypes.float8_e4m3
            )
            m[f"xt{s + 1}"] = np.ascontiguousarray(x[i].T).astype(np.float16)
        in_maps.append(m)
    res = run_bass_kernel_spmd(nc, in_maps, list(range(NCORES)))
    LAST_RESULT = res
    return np.stack([r["out"].astype(np.float32) for r in res.results]).reshape(
        b, n, c
    )


# revision 10
# speedup vs baseline: 1.7458x; 1.1091x over previous
"""JumpAttention (channel attention, cross-swapped values) on 8 trn2 cores.

Math (per batch b, per head h, hd=64):
  q,k,v = heads(x @ Wq), heads(x @ Wk), heads(x @ Wv)   laid out [hd, N]
  G_h   = q_h k_h^T (contraction over N)  ==  Wq_h^T (x^T x) Wk_h
  attn  = softmax(G / (||q||_clamped ||k||_clamped) * temp, axis=d)
  y1+y2 = x2 @ F1 + x1 @ F2,  F_s = concat_h(Wv_h @ attn_s_h^T)

Kernel structure (v2 — fp8 pass-1 + host-transposed x for pass-2):
  pass 1: S_s = x_s^T x_s from an fp8(e4m3) copy of x using DoubleRow
          matmuls (2 tokens packed per partition -> 2x fp16 PE rate).
          S is symmetric, so the lower-left 128x128 block is never
          computed; the interlude reads it through the transposed
          upper-right block (lhsT substitution).
  interlude (tiny, once): T=S@W, G=Wq^T T, norms via ones-matmul of
          W*T, softmax, F_s = Wv @ attn_s^T
  pass 2: out = x2 @ F1 + x1 @ F2 with lhsT streamed from a HOST-
          TRANSPOSED fp16 x^T in DRAM (no PE transposes at all),
          fp16 output written back (host upcasts to f32).

  DMA traffic/core: 8MB (x fp8) + 16MB (x^T fp16) + 8MB (out) = 32MB,
  all on one in-order SP queue ordered x8 -> xt -> out so the DMA
  engines (the bottleneck resource) never idle.

Sharding: pure data-parallel over B (B=8 == n_cores). No collectives.
"""

import os
import sys
from contextlib import ExitStack

import numpy as np
import ml_dtypes

for _p in ("/opt/trn_rl_repo",):
    if _p not in sys.path and os.path.isdir(_p):
        sys.path.insert(0, _p)

import concourse.bass as bass  # noqa: E402
import concourse.tile as tile  # noqa: E402
from concourse import bacc, mybir  # noqa: E402
from concourse.bass_utils import run_bass_kernel_spmd  # noqa: E402

B, N_FULL, C = 8, 16384, 256
H, HD = 4, 64
NCORES = 8

F32 = mybir.dt.float32
F16 = mybir.dt.float16
F8 = mybir.dt.float8e4
AF = mybir.ActivationFunctionType
DR = mybir.MatmulPerfMode.DoubleRow
EPS = 1e-12

SLABT = 4096  # tokens per x8 slab (1MB fp8)
WINT = 2048  # tokens per xt window (4 x 512KB fp16 tiles)
OSLT = 2048  # tokens per out slab (1MB fp16)
XT_BUFS = 6  # xt windows in flight per (stream, chunk)
BLOBW = 2176  # wkq pairs [0:1024], identity [1024:1152], wvt [1152:2176]


def _build(n_tokens: int):
    """Build + compile the single-core program (SPMD across 8 cores)."""
    nc = bacc.Bacc(
        "TRN2", target_bir_lowering=False, debug=False, num_devices=NCORES
    )
    x8 = [
        nc.dram_tensor(f"x{s + 1}f8", [n_tokens, C], F8, kind="ExternalInput").ap()
        for s in range(2)
    ]
    xt = [
        nc.dram_tensor(f"xt{s + 1}", [C, n_tokens], F16, kind="ExternalInput").ap()
        for s in range(2)
    ]
    wblob = nc.dram_tensor("wblob", [128, BLOBW], F16, kind="ExternalInput").ap()
    tmpd = nc.dram_tensor("tmpd", [128, 4], F32, kind="ExternalInput").ap()
    out = nc.dram_tensor("out", [n_tokens, C], F16, kind="ExternalOutput").ap()

    with tile.TileContext(nc) as tc, ExitStack() as ctx:
        _kernel(ctx, tc, out, x8, xt, wblob, tmpd, n_tokens)
    nc.compile()
    return nc


def _kernel(ctx, tc, out, x8in, xtin, wblob, tmpd, nt):
    nc = tc.nc
    nslab = nt // SLABT
    nwin = nt // WINT
    singles = ctx.enter_context(tc.tile_pool(name="singles", bufs=1))

    # ---- constants / weights to SBUF ----
    # blob cols: [0:1024] wkq pairs ([Wk|Wq] per 128-row chunk),
    # [1024:1152] identity, [1152:2176] wvt heads (rows 0-63).
    # wq/wk slices alias into the wkq region (no duplicate data).
    blob_sb = singles.tile([128, BLOBW], F16, tag="blob", name="blob")
    wkq_sb = [blob_sb[:, r * 512 : (r + 1) * 512] for r in range(2)]
    wq_sb = [blob_sb[:, r * 512 + C : (r + 1) * 512] for r in range(2)]
    wk_sb = [blob_sb[:, r * 512 : r * 512 + C] for r in range(2)]
    ident_sb = blob_sb[:, 1024:1152]
    wvt_sb = [blob_sb[0:HD, 1152 + h * C : 1152 + (h + 1) * C] for h in range(H)]
    tmps_sb = singles.tile([128, 4], F32, tag="tmps", name="tmps")
    tmp_sb = [tmps_sb[:, 2 * s : 2 * s + 2] for s in range(2)]
    ones_col = singles.tile([128, 1], F16, tag="ones_col", name="ones_col")
    nc.vector.memset(ones_col[:], 1.0)
    ones_row = singles.tile([1, 128], F16, tag="ones_row", name="ones_row")
    nc.vector.memset(ones_row[:], 1.0)

    S_sb = [
        [singles.tile([128, C], F16, tag=f"ssb{s}{c}", name=f"ssb{s}{c}") for c in range(2)]
        for s in range(2)
    ]

    # ================= pass 1: S = x^T x via fp8 DoubleRow =================
    # slab layout: partition p holds SLABT/128 consecutive tokens; a pair
    # tile tau contracts tokens {p*(SLABT/128) + 2*tau + j} over (p, j).
    x8_pool = ctx.enter_context(tc.tile_pool(name="x8", bufs=5))
    tpp = SLABT // 128  # tokens per partition per slab
    npair = tpp // 2  # pair tiles per slab
    # S row-chunk widths (symmetry: lower-left 128-block never computed)
    # chunk m0=0..3 covers S rows [m0*64, m0*64+64); rhs col range below.
    chunk_cols = [(0, 256), (0, 256), (128, 128), (128, 128)]

    # -- DMA issue order on the single in-order SP queue decides the DMA-
    # engine schedule: x8 slab 0 first (PE primes earliest), then consts,
    # x8 slabs 1-4, the rest of x8 as its rotating buffers free up, then
    # xt windows; out slabs queue last so the input stream is never delayed
    # and the DMA engines stay saturated start to finish.
    X8_BUFS = 5
    nslabs_total = 2 * nslab

    def issue_x8(k):
        s, si = divmod(k, nslab)
        sl = x8_pool.tile([128, SLABT * C // 128], F8, tag="x8slab", name="x8slab")
        nc.sync.dma_start(
            out=sl[:].rearrange("p (q c) -> p q c", q=tpp),
            in_=x8in[s][si * SLABT : (si + 1) * SLABT, :].rearrange(
                "(p q) c -> p q c", p=128
            ),
        )
        return sl

    x8_tiles = [issue_x8(0)]
    nc.sync.dma_start(out=blob_sb[:], in_=wblob[:, :])
    nc.sync.dma_start(out=tmps_sb[:], in_=tmpd[:, :])
    for k in range(1, X8_BUFS):
        x8_tiles.append(issue_x8(k))

    xt_pool = ctx.enter_context(tc.tile_pool(name="xt", bufs=XT_BUFS))
    xt_sb = {}

    def issue_xt_window(w):
        for s in range(2):
            for jc in range(2):
                xtile = xt_pool.tile(
                    [128, WINT], F16, tag=f"xt{s}{jc}", name=f"xt{s}{jc}"
                )
                nc.sync.dma_start(
                    out=xtile[:],
                    in_=xtin[s][jc * 128 : (jc + 1) * 128, w * WINT : (w + 1) * WINT],
                )
                xt_sb[s, jc, w] = xtile

    with ExitStack() as p1:
        psS = p1.enter_context(tc.tile_pool(name="psS", bufs=1, space="PSUM"))
        S_ps = [
            [psS.tile([128, C], F32, tag=f"s{s}{c}", name=f"s{s}{c}") for c in range(2)]
            for s in range(2)
        ]
        for k in range(nslabs_total):
            s, si = divmod(k, nslab)
            if True:
                sl = x8_tiles[k]
                for t in range(npair):
                    ti = si * npair + t
                    pv = sl[:, t * 2 * C : (t + 1) * 2 * C].rearrange(
                        "p (j c) -> p j c", j=2
                    )
                    for m0 in range(4):
                        c0, cw = chunk_cols[m0]
                        nc.tensor.matmul(
                            S_ps[s][m0 // 2][(m0 % 2) * 64 : (m0 % 2) * 64 + 64, c0 : c0 + cw],
                            lhsT=pv[:, :, m0 * 64 : (m0 + 1) * 64],
                            rhs=pv[:, :, c0 : c0 + cw],
                            start=(ti == 0),
                            stop=(ti == nslab * npair - 1),
                            perf_mode=DR,
                            skip_group_check=True,
                        )
                if k + X8_BUFS < nslabs_total:
                    x8_tiles.append(issue_x8(k + X8_BUFS))
        # xt windows 0..XT_BUFS-1 into fresh buffers; later windows are
        # issued inside the pass-2 loop once their buffer's readers exist.
        for w in range(min(XT_BUFS, nwin)):
            issue_xt_window(w)
        # S -> SBUF. tile0 rows 0:128 cols 0:256 full; tile1 cols 128:256 only.
        for s in range(2):
            if s == 0:
                nc.vector.tensor_copy(S_sb[s][0][:], S_ps[s][0][:])
                nc.vector.tensor_copy(
                    S_sb[s][1][:, 128:256], S_ps[s][1][:, 128:256]
                )
            else:
                nc.scalar.activation(S_sb[s][0][:], S_ps[s][0][:], AF.Copy)
                nc.scalar.activation(
                    S_sb[s][1][:, 128:256], S_ps[s][1][:, 128:256], AF.Copy
                )

    # ================= interlude =================
    # lhsT substitution for the never-computed lower-left S block:
    # S[128:256, 0:128] == S[0:128, 128:256]^T, and matmul transposes lhsT.
    def s_lhsT(s, jc, ic, cc_lo, cc_hi):
        if jc == 1 and ic == 0:
            return S_sb[s][0][:, 128 + cc_lo : 128 + cc_hi]
        return S_sb[s][jc][:, ic * 128 + cc_lo : ic * 128 + cc_hi]

    F_sb = [
        [singles.tile([128, C], F16, tag=f"f{s}{jc}", name=f"f{s}{jc}") for jc in range(2)]
        for s in range(2)
    ]
    with ExitStack() as il:
        big = il.enter_context(tc.tile_pool(name="ilbig", bufs=6, space="PSUM"))
        small = il.enter_context(
            tc.tile_pool(name="ilsmall", bufs=2, space="PSUM")
        )
        sb = il.enter_context(tc.tile_pool(name="ilsb", bufs=1))

        warm = sb.tile([1, 1], F32, tag="warm", name="warm")

        # -- stage A: [T_k | T_q] = S @ [Wk | Wq]  (f16 matmuls, N=512) --
        t_sb = {}
        for s in range(2):
            for ic in range(2):
                tp = big.tile([128, 2 * C], F32, tag="ilbig", name="ilbig")
                for jc in range(2):
                    nc.tensor.matmul(
                        tp[:],
                        lhsT=s_lhsT(s, jc, ic, 0, 128),
                        rhs=wkq_sb[jc],
                        start=(jc == 0),
                        stop=(jc == 1),
                    )
                st = sb.tile([128, 2 * C], F16, tag=f"t{ic}_{s}", name=f"t{ic}_{s}")
                if s == 0:
                    nc.vector.tensor_copy(st[:, 0:C], tp[:, 0:C])
                    nc.vector.tensor_copy(st[:, C : 2 * C], tp[:, C : 2 * C])
                else:
                    nc.scalar.activation(st[:], tp[:], AF.Copy)
                t_sb["k", s, ic] = st[:, 0:C]
                t_sb["q", s, ic] = st[:, C : 2 * C]

        # -- stage B: G = Wq^T @ T_k (psum) ; U = W * T (DVE) --
        g_ps = {}
        for s in range(2):
            for cc in range(2):
                g = big.tile([128, C], F32, tag="ilbig", name="ilbig")
                for ic in range(2):
                    nc.tensor.matmul(
                        g[:],
                        lhsT=wq_sb[ic][:, cc * 128 : (cc + 1) * 128],
                        rhs=t_sb["k", s, ic],
                        start=(ic == 0),
                        stop=(ic == 1),
                    )
                g_ps[s, cc] = g
        u_sb = {}
        for s in range(2):
            for nm in ("q", "k"):
                w_sb = wq_sb if nm == "q" else wk_sb
                for ic in range(2):
                    u = sb.tile([128, C], F16, tag=f"u{nm}{ic}_{s}", name=f"u{nm}{ic}_{s}")
                    nc.vector.tensor_mul(u[:], w_sb[ic], t_sb[nm, s, ic])
                    u_sb[nm, s, ic] = u

        # preload Sqrt table: emitted after every interlude ACT Copy so the
        # in-order ACT queue does Copy* -> Sqrt* with a single table load
        nc.scalar.activation(warm[:], S_sb[1][1][0:1, 128:129], AF.Sqrt)

        # -- stage C: norm matmuls + invq/invk (Sqrt clustered on ACT) --
        nq_ps, nk_ps = {}, {}
        for s in range(2):
            for cc in range(2):
                nq = small.tile([128, 1], F32, tag="ilsmall", name="ilsmall")
                for ic in range(2):
                    nc.tensor.matmul(
                        nq[:],
                        lhsT=u_sb["q", s, ic][:, cc * 128 : (cc + 1) * 128],
                        rhs=ones_col[:],
                        start=(ic == 0),
                        stop=(ic == 1),
                    )
                nq_ps[s, cc] = nq
            nk = small.tile([1, C], F32, tag="ilsmall", name="ilsmall_r")
            for ic in range(2):
                nc.tensor.matmul(
                    nk[:],
                    lhsT=ones_col[:],
                    rhs=u_sb["k", s, ic],
                    start=(ic == 0),
                    stop=(ic == 1),
                )
            nk_ps[s] = nk
        invq, ik16 = {}, {}
        for s in range(2):
            for cc in range(2):
                iq = sb.tile([128, 1], F32, tag=f"invq{cc}_{s}", name=f"invq{cc}_{s}")
                nc.scalar.activation(iq[:], nq_ps[s, cc][:], AF.Sqrt)
                invq[s, cc] = iq
            ik = sb.tile([1, C], F32, tag=f"invk_{s}", name=f"invk_{s}")
            nc.scalar.activation(ik[:], nk_ps[s][:], AF.Sqrt)
            ik16[s] = ik
        for s in range(2):
            for cc in range(2):
                iq = invq[s, cc]
                nc.vector.tensor_scalar_max(iq[:], iq[:], EPS)
                nc.vector.reciprocal(iq[:], iq[:])
                nc.vector.tensor_mul(iq[:], iq[:], tmp_sb[s][:, cc : cc + 1])
            ik = ik16[s]
            nc.vector.tensor_scalar_max(ik[:], ik[:], EPS)
            nc.vector.reciprocal(ik[:], ik[:])
            ikf = sb.tile([1, C], F16, tag=f"invk16_{s}", name=f"invk16_{s}")
            nc.vector.tensor_copy(ikf[:], ik[:])
            ik16[s] = ikf

        # preload Exp table while stage D runs on DVE
        nc.scalar.activation(warm[:], invq[0, 0][0:1, :], AF.Exp)

        # -- stage D: invk broadcast + logits assembly + row max --
        bc_ps, lp_sb, nm_sb = {}, {}, {}
        for s in range(2):
            bc = big.tile([128, C], F32, tag="ilbig", name="ilbig")
            nc.tensor.matmul(
                bc[:], lhsT=ones_row[:], rhs=ik16[s][:], start=True, stop=True
            )
            bc_ps[s] = bc
        for s in range(2):
            for cc in range(2):
                lp = sb.tile([128, HD], F32, tag=f"lp{cc}_{s}", name=f"lp{cc}_{s}")
                for half in range(2):
                    h = 2 * cc + half
                    rs = slice(half * 64, (half + 1) * 64)
                    cs = slice(h * 64, (h + 1) * 64)
                    nc.vector.tensor_scalar_mul(
                        lp[rs, :], g_ps[s, cc][rs, cs], invq[s, cc][rs, :]
                    )
                    nc.vector.tensor_mul(lp[rs, :], lp[rs, :], bc_ps[s][rs, cs])
                nmt = sb.tile([128, 1], F32, tag=f"nm{cc}_{s}", name=f"nm{cc}_{s}")
                nc.vector.tensor_reduce(
                    nmt[:], lp[:], axis=mybir.AxisListType.X,
                    op=mybir.AluOpType.max, negate=True,
                )
                lp_sb[s, cc] = lp
                nm_sb[s, cc] = nmt

        # -- stage E: Exp (clustered) + normalize --
        attn16 = {}
        for s in range(2):
            for cc in range(2):
                pexp = sb.tile([128, HD], F32, tag=f"pexp{cc}_{s}", name=f"pexp{cc}_{s}")
                sm = sb.tile([128, 1], F32, tag=f"sm{cc}_{s}", name=f"sm{cc}_{s}")
                nc.scalar.activation(
                    pexp[:], lp_sb[s, cc][:], AF.Exp,
                    bias=nm_sb[s, cc][:], accum_out=sm[:],
                )
                attn16[s, cc] = (pexp, sm)
        for s in range(2):
            for cc in range(2):
                pexp, sm = attn16[s, cc]
                nc.vector.reciprocal(sm[:], sm[:])
                a16 = sb.tile([128, HD], F16, tag=f"a16{cc}_{s}", name=f"a16{cc}_{s}")
                nc.vector.tensor_scalar_mul(a16[:], pexp[:], sm[:])
                attn16[s, cc] = a16

        # -- stage F: attn^T + F = concat_h(Wv_h @ attn_h^T) --
        at_sb = {}
        for s in range(2):
            for cc in range(2):
                atp = small.tile([HD, 128], F16, tag="ilsmall", name="ilsmall_t")
                nc.tensor.transpose(atp[:], attn16[s, cc][:], ident_sb)
                at = sb.tile([HD, 128], F16, tag=f"at{cc}_{s}", name=f"at{cc}_{s}")
                nc.vector.tensor_copy(at[:], atp[:])
                at_sb[s, cc] = at
        for s in range(2):
            for jc in range(2):
                fp = big.tile([128, C], F32, tag="ilbig", name="ilbig")
                for h in range(H):
                    cc, half = divmod(h, 2)
                    nc.tensor.matmul(
                        fp[:, h * 64 : (h + 1) * 64],
                        lhsT=wvt_sb[h][:, jc * 128 : (jc + 1) * 128],
                        rhs=at_sb[s, cc][:, half * 64 : (half + 1) * 64],
                        start=True,
                        stop=True,
                    )
                if s == 0:
                    nc.vector.tensor_copy(F_sb[s][jc][:], fp[:])
                else:
                    nc.scalar.activation(F_sb[s][jc][:], fp[:], AF.Copy)

    # ================= pass 2: out = x2@F1 + x1@F2 =================
    with ExitStack() as p2:
        psO = p2.enter_context(tc.tile_pool(name="psO", bufs=6, space="PSUM"))
        opool = p2.enter_context(tc.tile_pool(name="opool", bufs=6))
        # pairs: (xt stream index, F): y1 = x2@F1, y2 = x1@F2
        pairs = [(1, F_sb[0]), (0, F_sb[1])]
        tpo = WINT // 128  # psum tiles per window / out slab
        for w in range(nwin):
            osl = opool.tile([128, tpo * C], F16, tag="osl", name="osl")
            for t in range(tpo):
                op = psO.tile([128, C], F32, tag="op", name="op")
                idx = 0
                for sx, fs in pairs:
                    for jc in range(2):
                        nc.tensor.matmul(
                            op[:],
                            lhsT=xt_sb[sx, jc, w][:, t * 128 : (t + 1) * 128],
                            rhs=fs[jc][:],
                            start=(idx == 0),
                            stop=(idx == 3),
                        )
                        idx += 1
                if t % 2 == 0:
                    nc.vector.tensor_copy(osl[:, t * C : (t + 1) * C], op[:])
                else:
                    nc.scalar.activation(
                        osl[:, t * C : (t + 1) * C], op[:], AF.Copy
                    )
            nc.sync.dma_start(
                out=out[w * WINT : (w + 1) * WINT, :].rearrange(
                    "(t p) c -> p t c", p=128
                ),
                in_=osl[:].rearrange("p (t c) -> p t c", t=tpo),
            )
            if w + XT_BUFS < nwin:
                # late xt windows queue behind this out slab; their rotating
                # buffer's readers (window w's matmuls) are now emitted.
                issue_xt_window(w + XT_BUFS)


def _host_prep(w_qkv, temperature, temperature2):
    w = np.asarray(w_qkv, dtype=np.float32)
    wq = w[:, 0:C].astype(np.float16)
    wk = w[:, C : 2 * C].astype(np.float16)
    wvt = np.ascontiguousarray(w[:, 2 * C : 3 * C].T.reshape(H, HD, C)).astype(
        np.float16
    )
    blob = np.zeros((128, BLOBW), dtype=np.float16)
    for r in range(2):
        blob[:, r * 512 : r * 512 + C] = wk[r * 128 : (r + 1) * 128, :]
        blob[:, r * 512 + C : (r + 1) * 512] = wq[r * 128 : (r + 1) * 128, :]
    blob[:, 1024:1152] = np.eye(128, dtype=np.float16)
    for h in range(H):
        blob[0:HD, 1152 + h * C : 1152 + (h + 1) * C] = wvt[h]
    tmp = []
    for tarr in (temperature, temperature2):
        t = np.asarray(tarr, dtype=np.float32).reshape(H)
        tmp.append(
            np.stack(
                [np.repeat(t[[0, 1]], 64), np.repeat(t[[2, 3]], 64)], axis=1
            ).astype(np.float32)
        )
    tmpd = np.concatenate(tmp, axis=1).astype(np.float32)
    return blob, tmpd


_NC_CACHE = {}
LAST_RESULT = None


def _get_nc(n_tokens):
    if n_tokens not in _NC_CACHE:
        _NC_CACHE[n_tokens] = _build(n_tokens)
    return _NC_CACHE[n_tokens]


def kernel(x1, x2, w_qkv, temperature, temperature2):
    global LAST_RESULT
    x1 = np.asarray(x1, dtype=np.float32)
    x2 = np.asarray(x2, dtype=np.float32)
    b, n, c = x1.shape
    assert c == C and b == NCORES, (b, n, c)
    wblob, tmpd = _host_prep(w_qkv, temperature, temperature2)
    nc = _get_nc(n)
    in_maps = []
    for i in range(NCORES):
        m = {"wblob": wblob, "tmpd": tmpd}
        for s, x in enumerate((x1, x2)):
            m[f"x{s + 1}f8"] = np.ascontiguousarray(x[i]).astype(
                ml_dtypes.float8_e4m3
            )
            m[f"xt{s + 1}"] = np.ascontiguousarray(x[i].T).astype(np.float16)
        in_maps.append(m)
    res = run_bass_kernel_spmd(nc, in_maps, list(range(NCORES)))
    LAST_RESULT = res
    return np.stack([r["out"].astype(np.float32) for r in res.results]).reshape(
        b, n, c
    )


# revision 11
# speedup vs baseline: 1.7540x; 1.0047x over previous
"""JumpAttention (channel attention, cross-swapped values) on 8 trn2 cores.

Math (per batch b, per head h, hd=64):
  q,k,v = heads(x @ Wq), heads(x @ Wk), heads(x @ Wv)   laid out [hd, N]
  G_h   = q_h k_h^T (contraction over N)  ==  Wq_h^T (x^T x) Wk_h
  attn  = softmax(G / (||q||_clamped ||k||_clamped) * temp, axis=d)
  y1+y2 = x2 @ F1 + x1 @ F2,  F_s = concat_h(Wv_h @ attn_s_h^T)

Kernel structure (v2 — fp8 pass-1 + host-transposed x for pass-2):
  pass 1: S_s = x_s^T x_s from an fp8(e4m3) copy of x using DoubleRow
          matmuls (2 tokens packed per partition -> 2x fp16 PE rate).
          S is symmetric, so the lower-left 128x128 block is never
          computed; the interlude reads it through the transposed
          upper-right block (lhsT substitution).
  interlude (tiny, once): T=S@W, G=Wq^T T, norms via ones-matmul of
          W*T, softmax, F_s = Wv @ attn_s^T
  pass 2: out = x2 @ F1 + x1 @ F2 with lhsT streamed from a HOST-
          TRANSPOSED fp16 x^T in DRAM (no PE transposes at all),
          fp16 output written back (host upcasts to f32).

  DMA traffic/core: 8MB (x fp8) + 16MB (x^T fp16) + 8MB (out) = 32MB,
  all on one in-order SP queue ordered x8 -> xt -> out so the DMA
  engines (the bottleneck resource) never idle.

Sharding: pure data-parallel over B (B=8 == n_cores). No collectives.
"""

import os
import sys
from contextlib import ExitStack

import numpy as np
import ml_dtypes

for _p in ("/opt/trn_rl_repo",):
    if _p not in sys.path and os.path.isdir(_p):
        sys.path.insert(0, _p)

import concourse.bass as bass  # noqa: E402
import concourse.tile as tile  # noqa: E402
from concourse import bacc, mybir  # noqa: E402
from concourse.bass_utils import run_bass_kernel_spmd  # noqa: E402

B, N_FULL, C = 8, 16384, 256
H, HD = 4, 64
NCORES = 8

F32 = mybir.dt.float32
F16 = mybir.dt.float16
F8 = mybir.dt.float8e4
AF = mybir.ActivationFunctionType
DR = mybir.MatmulPerfMode.DoubleRow
EPS = 1e-12

SLABT = 4096  # tokens per x8 slab (1MB fp8)
WINT = 2048  # tokens per xt window (4 x 512KB fp16 tiles)
OSLT = 2048  # tokens per out slab (1MB fp16)
XT_BUFS = 6  # xt windows in flight per (stream, chunk)
BLOBW = 2176  # wkq pairs [0:1024], identity [1024:1152], wvt [1152:2176]


def _build(n_tokens: int):
    """Build + compile the single-core program (SPMD across 8 cores)."""
    nc = bacc.Bacc(
        "TRN2", target_bir_lowering=False, debug=False, num_devices=NCORES
    )
    x8 = [
        nc.dram_tensor(f"x{s + 1}f8", [n_tokens, C], F8, kind="ExternalInput").ap()
        for s in range(2)
    ]
    xt = [
        nc.dram_tensor(f"xt{s + 1}", [C, n_tokens], F16, kind="ExternalInput").ap()
        for s in range(2)
    ]
    wblob = nc.dram_tensor("wblob", [128, BLOBW], F16, kind="ExternalInput").ap()
    tmpd = nc.dram_tensor("tmpd", [128, 4], F32, kind="ExternalInput").ap()
    out = nc.dram_tensor("out", [n_tokens, C], F16, kind="ExternalOutput").ap()

    with tile.TileContext(nc) as tc, ExitStack() as ctx:
        _kernel(ctx, tc, out, x8, xt, wblob, tmpd, n_tokens)
    nc.compile()
    return nc


def _kernel(ctx, tc, out, x8in, xtin, wblob, tmpd, nt):
    nc = tc.nc
    nslab = nt // SLABT
    nwin = nt // WINT
    singles = ctx.enter_context(tc.tile_pool(name="singles", bufs=1))

    # ---- constants / weights to SBUF ----
    # blob cols: [0:1024] wkq pairs ([Wk|Wq] per 128-row chunk),
    # [1024:1152] identity, [1152:2176] wvt heads (rows 0-63).
    # wq/wk slices alias into the wkq region (no duplicate data).
    blob_sb = singles.tile([128, BLOBW], F16, tag="blob", name="blob")
    wkq_sb = [blob_sb[:, r * 512 : (r + 1) * 512] for r in range(2)]
    wq_sb = [blob_sb[:, r * 512 + C : (r + 1) * 512] for r in range(2)]
    wk_sb = [blob_sb[:, r * 512 : r * 512 + C] for r in range(2)]
    ident_sb = blob_sb[:, 1024:1152]
    wvt_sb = [blob_sb[0:HD, 1152 + h * C : 1152 + (h + 1) * C] for h in range(H)]
    tmps_sb = singles.tile([128, 4], F32, tag="tmps", name="tmps")
    tmp_sb = [tmps_sb[:, 2 * s : 2 * s + 2] for s in range(2)]
    ones_col = singles.tile([128, 1], F16, tag="ones_col", name="ones_col")
    nc.vector.memset(ones_col[:], 1.0)
    ones_row = singles.tile([1, 128], F16, tag="ones_row", name="ones_row")
    nc.vector.memset(ones_row[:], 1.0)

    S_sb = [
        [singles.tile([128, C], F16, tag=f"ssb{s}{c}", name=f"ssb{s}{c}") for c in range(2)]
        for s in range(2)
    ]

    # ================= pass 1: S = x^T x via fp8 DoubleRow =================
    # slab layout: partition p holds SLABT/128 consecutive tokens; a pair
    # tile tau contracts tokens {p*(SLABT/128) + 2*tau + j} over (p, j).
    x8_pool = ctx.enter_context(tc.tile_pool(name="x8", bufs=5))
    tpp = SLABT // 128  # tokens per partition per slab
    npair = tpp // 2  # pair tiles per slab
    # S row-chunk widths (symmetry: lower-left 128-block never computed)
    # chunk m0=0..3 covers S rows [m0*64, m0*64+64); rhs col range below.
    chunk_cols = [(0, 256), (0, 256), (128, 128), (128, 128)]

    # -- DMA issue order on the single in-order SP queue decides the DMA-
    # engine schedule: x8 slab 0 first (PE primes earliest), then consts,
    # x8 slabs 1-4, the rest of x8 as its rotating buffers free up, then
    # xt windows; out slabs queue last so the input stream is never delayed
    # and the DMA engines stay saturated start to finish.
    X8_BUFS = 5
    nslabs_total = 2 * nslab

    def issue_x8(k):
        s, si = divmod(k, nslab)
        sl = x8_pool.tile([128, SLABT * C // 128], F8, tag="x8slab", name="x8slab")
        nc.sync.dma_start(
            out=sl[:].rearrange("p (q c) -> p q c", q=tpp),
            in_=x8in[s][si * SLABT : (si + 1) * SLABT, :].rearrange(
                "(p q) c -> p q c", p=128
            ),
        )
        return sl

    x8_tiles = [issue_x8(0)]
    nc.sync.dma_start(out=blob_sb[:], in_=wblob[:, :])
    nc.sync.dma_start(out=tmps_sb[:], in_=tmpd[:, :])
    for k in range(1, X8_BUFS):
        x8_tiles.append(issue_x8(k))

    xt_pool = ctx.enter_context(tc.tile_pool(name="xt", bufs=XT_BUFS))
    xt_sb = {}

    def issue_xt_window(w):
        for s in range(2):
            for jc in range(2):
                xtile = xt_pool.tile(
                    [128, WINT], F16, tag=f"xt{s}{jc}", name=f"xt{s}{jc}"
                )
                nc.sync.dma_start(
                    out=xtile[:],
                    in_=xtin[s][jc * 128 : (jc + 1) * 128, w * WINT : (w + 1) * WINT],
                )
                xt_sb[s, jc, w] = xtile

    with ExitStack() as p1:
        psS = p1.enter_context(tc.tile_pool(name="psS", bufs=1, space="PSUM"))
        S_ps = [
            [psS.tile([128, C], F32, tag=f"s{s}{c}", name=f"s{s}{c}") for c in range(2)]
            for s in range(2)
        ]
        for k in range(nslabs_total):
            s, si = divmod(k, nslab)
            if True:
                sl = x8_tiles[k]
                for t in range(npair):
                    ti = si * npair + t
                    pv = sl[:, t * 2 * C : (t + 1) * 2 * C].rearrange(
                        "p (j c) -> p j c", j=2
                    )
                    for m0 in range(4):
                        c0, cw = chunk_cols[m0]
                        nc.tensor.matmul(
                            S_ps[s][m0 // 2][(m0 % 2) * 64 : (m0 % 2) * 64 + 64, c0 : c0 + cw],
                            lhsT=pv[:, :, m0 * 64 : (m0 + 1) * 64],
                            rhs=pv[:, :, c0 : c0 + cw],
                            start=(ti == 0),
                            stop=(ti == nslab * npair - 1),
                            perf_mode=DR,
                            skip_group_check=True,
                        )
                if k + X8_BUFS < nslabs_total:
                    x8_tiles.append(issue_x8(k + X8_BUFS))
        # xt windows 0..XT_BUFS-1 into fresh buffers; later windows are
        # issued inside the pass-2 loop once their buffer's readers exist.
        for w in range(min(XT_BUFS, nwin)):
            issue_xt_window(w)
        # S -> SBUF. tile0 rows 0:128 cols 0:256 full; tile1 cols 128:256 only.
        for s in range(2):
            if s == 0:
                nc.vector.tensor_copy(S_sb[s][0][:], S_ps[s][0][:])
                nc.vector.tensor_copy(
                    S_sb[s][1][:, 128:256], S_ps[s][1][:, 128:256]
                )
            else:
                nc.scalar.activation(S_sb[s][0][:], S_ps[s][0][:], AF.Copy)
                nc.scalar.activation(
                    S_sb[s][1][:, 128:256], S_ps[s][1][:, 128:256], AF.Copy
                )

    # ================= interlude =================
    # lhsT substitution for the never-computed lower-left S block:
    # S[128:256, 0:128] == S[0:128, 128:256]^T, and matmul transposes lhsT.
    def s_lhsT(s, jc, ic, cc_lo, cc_hi):
        if jc == 1 and ic == 0:
            return S_sb[s][0][:, 128 + cc_lo : 128 + cc_hi]
        return S_sb[s][jc][:, ic * 128 + cc_lo : ic * 128 + cc_hi]

    F_sb = [
        [singles.tile([128, C], F16, tag=f"f{s}{jc}", name=f"f{s}{jc}") for jc in range(2)]
        for s in range(2)
    ]
    with ExitStack() as il:
        big = il.enter_context(tc.tile_pool(name="ilbig", bufs=6, space="PSUM"))
        small = il.enter_context(
            tc.tile_pool(name="ilsmall", bufs=2, space="PSUM")
        )
        sb = il.enter_context(tc.tile_pool(name="ilsb", bufs=1))

        warm = sb.tile([1, 1], F32, tag="warm", name="warm")

        # -- stage A: [T_k | T_q] = S @ [Wk | Wq]  (f16 matmuls, N=512) --
        t_sb = {}
        for s in range(2):
            for ic in range(2):
                tp = big.tile([128, 2 * C], F32, tag="ilbig", name="ilbig")
                for jc in range(2):
                    nc.tensor.matmul(
                        tp[:],
                        lhsT=s_lhsT(s, jc, ic, 0, 128),
                        rhs=wkq_sb[jc],
                        start=(jc == 0),
                        stop=(jc == 1),
                    )
                st = sb.tile([128, 2 * C], F16, tag=f"t{ic}_{s}", name=f"t{ic}_{s}")
                if s == 0:
                    nc.vector.tensor_copy(st[:, 0:C], tp[:, 0:C])
                    nc.vector.tensor_copy(st[:, C : 2 * C], tp[:, C : 2 * C])
                else:
                    nc.scalar.activation(st[:], tp[:], AF.Copy)
                t_sb["k", s, ic] = st[:, 0:C]
                t_sb["q", s, ic] = st[:, C : 2 * C]

        # -- stage B: G = Wq^T @ T_k (psum) ; U = W * T (DVE) --
        g_ps = {}
        for s in range(2):
            for cc in range(2):
                g = big.tile([128, C], F32, tag="ilbig", name="ilbig")
                for ic in range(2):
                    nc.tensor.matmul(
                        g[:],
                        lhsT=wq_sb[ic][:, cc * 128 : (cc + 1) * 128],
                        rhs=t_sb["k", s, ic],
                        start=(ic == 0),
                        stop=(ic == 1),
                    )
                g_ps[s, cc] = g
        u_sb = {}
        for s in range(2):
            for nm in ("q", "k"):
                w_sb = wq_sb if nm == "q" else wk_sb
                for ic in range(2):
                    u = sb.tile([128, C], F16, tag=f"u{nm}{ic}_{s}", name=f"u{nm}{ic}_{s}")
                    nc.vector.tensor_mul(u[:], w_sb[ic], t_sb[nm, s, ic])
                    u_sb[nm, s, ic] = u

        # preload Sqrt table: emitted after every interlude ACT Copy so the
        # in-order ACT queue does Copy* -> Sqrt* with a single table load
        nc.scalar.activation(warm[:], S_sb[1][1][0:1, 128:129], AF.Sqrt)

        # -- stage C: norm matmuls + invq/invk (Sqrt clustered on ACT) --
        nq_ps, nk_ps = {}, {}
        for s in range(2):
            for cc in range(2):
                nq = small.tile([128, 1], F32, tag="ilsmall", name="ilsmall")
                for ic in range(2):
                    nc.tensor.matmul(
                        nq[:],
                        lhsT=u_sb["q", s, ic][:, cc * 128 : (cc + 1) * 128],
                        rhs=ones_col[:],
                        start=(ic == 0),
                        stop=(ic == 1),
                    )
                nq_ps[s, cc] = nq
            nk = small.tile([1, C], F32, tag="ilsmall", name="ilsmall_r")
            for ic in range(2):
                nc.tensor.matmul(
                    nk[:],
                    lhsT=ones_col[:],
                    rhs=u_sb["k", s, ic],
                    start=(ic == 0),
                    stop=(ic == 1),
                )
            nk_ps[s] = nk
        invq, ik16 = {}, {}
        for s in range(2):
            for cc in range(2):
                iq = sb.tile([128, 1], F32, tag=f"invq{cc}_{s}", name=f"invq{cc}_{s}")
                nc.scalar.activation(iq[:], nq_ps[s, cc][:], AF.Sqrt)
                invq[s, cc] = iq
            ik = sb.tile([1, C], F32, tag=f"invk_{s}", name=f"invk_{s}")
            nc.scalar.activation(ik[:], nk_ps[s][:], AF.Sqrt)
            ik16[s] = ik
        for s in range(2):
            for cc in range(2):
                iq = invq[s, cc]
                nc.vector.tensor_scalar_max(iq[:], iq[:], EPS)
                nc.vector.reciprocal(iq[:], iq[:])
                nc.vector.tensor_mul(iq[:], iq[:], tmp_sb[s][:, cc : cc + 1])
            ik = ik16[s]
            nc.vector.tensor_scalar_max(ik[:], ik[:], EPS)
            nc.vector.reciprocal(ik[:], ik[:])
            ikf = sb.tile([1, C], F16, tag=f"invk16_{s}", name=f"invk16_{s}")
            nc.vector.tensor_copy(ikf[:], ik[:])
            ik16[s] = ikf

        # preload Exp table while stage D runs on DVE
        nc.scalar.activation(warm[:], invq[0, 0][0:1, :], AF.Exp)

        # -- stage D: invk broadcast + logits assembly + row max --
        bc_ps, lp_sb, nm_sb = {}, {}, {}
        for s in range(2):
            bc = big.tile([128, C], F32, tag="ilbig", name="ilbig")
            nc.tensor.matmul(
                bc[:], lhsT=ones_row[:], rhs=ik16[s][:], start=True, stop=True
            )
            bc_ps[s] = bc
        for s in range(2):
            for cc in range(2):
                lp = sb.tile([128, HD], F32, tag=f"lp{cc}_{s}", name=f"lp{cc}_{s}")
                for half in range(2):
                    h = 2 * cc + half
                    rs = slice(half * 64, (half + 1) * 64)
                    cs = slice(h * 64, (h + 1) * 64)
                    nc.vector.tensor_scalar_mul(
                        lp[rs, :], g_ps[s, cc][rs, cs], invq[s, cc][rs, :]
                    )
                    nc.vector.tensor_mul(lp[rs, :], lp[rs, :], bc_ps[s][rs, cs])
                nmt = sb.tile([128, 1], F32, tag=f"nm{cc}_{s}", name=f"nm{cc}_{s}")
                nc.vector.tensor_reduce(
                    nmt[:], lp[:], axis=mybir.AxisListType.X,
                    op=mybir.AluOpType.max, negate=True,
                )
                lp_sb[s, cc] = lp
                nm_sb[s, cc] = nmt

        # -- stage E: Exp (clustered) + normalize --
        attn16 = {}
        for s in range(2):
            for cc in range(2):
                pexp = sb.tile([128, HD], F32, tag=f"pexp{cc}_{s}", name=f"pexp{cc}_{s}")
                sm = sb.tile([128, 1], F32, tag=f"sm{cc}_{s}", name=f"sm{cc}_{s}")
                nc.scalar.activation(
                    pexp[:], lp_sb[s, cc][:], AF.Exp,
                    bias=nm_sb[s, cc][:], accum_out=sm[:],
                )
                attn16[s, cc] = (pexp, sm)
        for s in range(2):
            for cc in range(2):
                pexp, sm = attn16[s, cc]
                nc.vector.reciprocal(sm[:], sm[:])
                a16 = sb.tile([128, HD], F16, tag=f"a16{cc}_{s}", name=f"a16{cc}_{s}")
                nc.vector.tensor_scalar_mul(a16[:], pexp[:], sm[:])
                attn16[s, cc] = a16

        # -- stage F: attn^T + F = concat_h(Wv_h @ attn_h^T) --
        at_sb = {}
        for s in range(2):
            for cc in range(2):
                atp = small.tile([HD, 128], F16, tag="ilsmall", name="ilsmall_t")
                nc.tensor.transpose(atp[:], attn16[s, cc][:], ident_sb)
                at = sb.tile([HD, 128], F16, tag=f"at{cc}_{s}", name=f"at{cc}_{s}")
                nc.vector.tensor_copy(at[:], atp[:])
                at_sb[s, cc] = at
        for s in range(2):
            for jc in range(2):
                fp = big.tile([128, C], F32, tag="ilbig", name="ilbig")
                for h in range(H):
                    cc, half = divmod(h, 2)
                    nc.tensor.matmul(
                        fp[:, h * 64 : (h + 1) * 64],
                        lhsT=wvt_sb[h][:, jc * 128 : (jc + 1) * 128],
                        rhs=at_sb[s, cc][:, half * 64 : (half + 1) * 64],
                        start=True,
                        stop=True,
                    )
                if s == 0:
                    nc.vector.tensor_copy(F_sb[s][jc][:], fp[:])
                else:
                    nc.scalar.activation(F_sb[s][jc][:], fp[:], AF.Copy)

    # ================= pass 2: out = x2@F1 + x1@F2 =================
    with ExitStack() as p2:
        psO = p2.enter_context(tc.tile_pool(name="psO", bufs=6, space="PSUM"))
        opool = p2.enter_context(tc.tile_pool(name="opool", bufs=6))
        # pairs: (xt stream index, F): y1 = x2@F1, y2 = x1@F2
        pairs = [(1, F_sb[0]), (0, F_sb[1])]
        tpo = WINT // 128  # psum tiles per window / out slab
        for w in range(nwin):
            # parity-split out staging: DVE owns even psum tiles, ACT odd —
            # separate SBUF tiles so the copies never serialize cross-engine.
            osl = [
                opool.tile([128, tpo // 2 * C], F16, tag=f"osl{par}", name=f"osl{par}")
                for par in range(2)
            ]
            for t in range(tpo):
                op = psO.tile([128, C], F32, tag="op", name="op")
                idx = 0
                for sx, fs in pairs:
                    for jc in range(2):
                        nc.tensor.matmul(
                            op[:],
                            lhsT=xt_sb[sx, jc, w][:, t * 128 : (t + 1) * 128],
                            rhs=fs[jc][:],
                            start=(idx == 0),
                            stop=(idx == 3),
                        )
                        idx += 1
                dst = osl[t % 2][:, t // 2 * C : (t // 2 + 1) * C]
                if t % 2 == 0:
                    nc.vector.tensor_copy(dst, op[:])
                else:
                    nc.scalar.activation(dst, op[:], AF.Copy)
            for par in range(2):
                nc.sync.dma_start(
                    out=out[w * WINT : (w + 1) * WINT, :].rearrange(
                        "(t two p) c -> two p t c", p=128, two=2
                    )[par],
                    in_=osl[par][:].rearrange("p (t c) -> p t c", t=tpo // 2),
                )
            if w + XT_BUFS < nwin:
                # late xt windows queue behind this out slab; their rotating
                # buffer's readers (window w's matmuls) are now emitted.
                issue_xt_window(w + XT_BUFS)


def _host_prep(w_qkv, temperature, temperature2):
    w = np.asarray(w_qkv, dtype=np.float32)
    wq = w[:, 0:C].astype(np.float16)
    wk = w[:, C : 2 * C].astype(np.float16)
    wvt = np.ascontiguousarray(w[:, 2 * C : 3 * C].T.reshape(H, HD, C)).astype(
        np.float16
    )
    blob = np.zeros((128, BLOBW), dtype=np.float16)
    for r in range(2):
        blob[:, r * 512 : r * 512 + C] = wk[r * 128 : (r + 1) * 128, :]
        blob[:, r * 512 + C : (r + 1) * 512] = wq[r * 128 : (r + 1) * 128, :]
    blob[:, 1024:1152] = np.eye(128, dtype=np.float16)
    for h in range(H):
        blob[0:HD, 1152 + h * C : 1152 + (h + 1) * C] = wvt[h]
    tmp = []
    for tarr in (temperature, temperature2):
        t = np.asarray(tarr, dtype=np.float32).reshape(H)
        tmp.append(
            np.stack(
                [np.repeat(t[[0, 1]], 64), np.repeat(t[[2, 3]], 64)], axis=1
            ).astype(np.float32)
        )
    tmpd = np.concatenate(tmp, axis=1).astype(np.float32)
    return blob, tmpd


_NC_CACHE = {}
LAST_RESULT = None


def _get_nc(n_tokens):
    if n_tokens not in _NC_CACHE:
        _NC_CACHE[n_tokens] = _build(n_tokens)
    return _NC_CACHE[n_tokens]


def kernel(x1, x2, w_qkv, temperature, temperature2):
    global LAST_RESULT
    x1 = np.asarray(x1, dtype=np.float32)
    x2 = np.asarray(x2, dtype=np.float32)
    b, n, c = x1.shape
    assert c == C and b == NCORES, (b, n, c)
    wblob, tmpd = _host_prep(w_qkv, temperature, temperature2)
    nc = _get_nc(n)
    in_maps = []
    for i in range(NCORES):
        m = {"wblob": wblob, "tmpd": tmpd}
        for s, x in enumerate((x1, x2)):
            m[f"x{s + 1}f8"] = np.ascontiguousarray(x[i]).astype(
                ml_dtypes.float8_e4m3
            )
            m[f"xt{s + 1}"] = np.ascontiguousarray(x[i].T).astype(np.float16)
        in_maps.append(m)
    res = run_bass_kernel_spmd(nc, in_maps, list(range(NCORES)))
    LAST_RESULT = res
    return np.stack([r["out"].astype(np.float32) for r in res.results]).reshape(
        b, n, c
    )
